# revision 1
# baseline (speedup 1.0000x reference)
"""GhostAttention (B=2, T=2048, C=2048, H=16) on 8 Trainium2 NeuronCores.

Sharding: tensor-parallel over heads (Megatron-style). Core c owns heads
{2c, 2c+1}: it gets the 256 matching rows of Wq/Wk/Wv (column-parallel) and
the 256 matching columns of Wo (row-parallel), computes QKV projections,
masked-relu attention and its partial output projection for both batches,
and writes a full-shape partial y. The host sums the 8 partials.

Per-core dataflow (all matmuls fp32r: fp32 storage, bf16-rate on the PE):
  phase 1: q,k in (hd, tok) layout and v in (tok, hd) layout, accumulating
           over 16 K-tiles of x^T streamed from HBM.
  phase 2: S^T blocks (tk=128, tq=512) = k-stationary @ q-moving; ACT applies
           relu(S + 0.1) draining PSUM->SBUF (scale folded into q); diagonal
           blocks get a causal 0/1 mask multiply on DVE; AV accumulates
           v-stationary @ w-moving into (hd, tq) PSUM; a ones-column matmul
           accumulates the normalizer; its reciprocal is broadcast across
           partitions with a rank-1 matmul and applied on DVE.
  phase 3: out-projection, attn-stationary @ Wo-moving -> (tok, o) PSUM,
           staged to SBUF and DMA'd to the partial output.
"""

import math
import sys

if "/opt/trn_rl_repo" not in sys.path:
    sys.path.insert(0, "/opt/trn_rl_repo")

import numpy as np
from contextlib import ExitStack

import concourse.bass as bass
import concourse.mybir as mybir
import concourse.tile as tile
from concourse.bass import ts, ds
from concourse.bass_utils import run_bass_kernel_spmd
from concourse.vector_clock import ScopedClock, VectorClock


def _split_drain_and_barrier(self, tick_clock, wait_clock):
    # This image's walrus caps sem waits per instruction; split the Tile-tail
    # drain waits across single-wait SP nops instead.
    gc = tick_clock.global_clock
    n = len(gc)
    for proc in range(n):
        t = gc[proc]
        if t <= 0:
            continue
        vc = VectorClock([0] * n)
        vc.require_at_least(proc, t)
        nop_inst = self.nc.sync.nop()
        wait_clock.add_sem_waits(nop_inst.ins, ScopedClock({None: vc}))
    self.nc.sync.drain()
    self.nc.all_engine_barrier()
    assert self.sems is not None
    popped = self.nc._tile_sem_poison_stack.pop()
    assert popped is self._sem_poison
    self.nc.clear_and_free_semaphores(list(self.sems.allocated().values()))
    self.nc.all_engine_barrier()


tile.TileContext._drain_and_barrier = _split_drain_and_barrier

_ws_counter = [0]


def split_excess_waits(nc, max_waits=1):
    """Hoist extra per-instruction sem waits onto preceding same-engine NoOps
    (same queue => they execute, and therefore wait, before the instruction)."""
    for fn in nc.m.functions:
        for blk in fn.blocks:
            insts = list(blk.instructions)
            out = []
            changed = False
            for inst in insts:
                si = inst.sync_info
                if si is not None and si.on_wait and len(si.on_wait) > max_waits:
                    waits = list(si.on_wait)
                    extra, keep = waits[:-max_waits], waits[-max_waits:]
                    for s in range(0, len(extra), max_waits):
                        chunk = extra[s : s + max_waits]
                        _ws_counter[0] += 1
                        nop = mybir.InstNoOp(
                            name=f"I-ws-{_ws_counter[0]}",
                            engine=inst.engine,
                            ins=[],
                            outs=[],
                            sync_info=mybir.SyncInfo(on_wait=chunk, on_update=[]),
                        )
                        out.append(nop)
                    inst.sync_info = mybir.SyncInfo(
                        on_wait=keep, on_update=list(si.on_update)
                    )
                    changed = True
                out.append(inst)
            if changed:
                try:
                    blk.instructions[:] = out
                except Exception:
                    blk.set_instructions(out)
    return nc


B, T, C = 2, 2048, 2048
H = 16
HD = C // H  # 128
N_CORES = 8
H_PER_CORE = H // N_CORES  # 2
CH = HD * H_PER_CORE  # 256 channels per core
SCALE = 1.0 / math.sqrt(HD)
ATTN_BIAS = 0.1  # relu(scores - (-0.1)) = relu(scores + 0.1)
EPS = 1e-6

F32 = mybir.dt.float32
F32R = mybir.dt.float32r
AF = mybir.ActivationFunctionType

_NC_CACHE = None


def _build(split_waits=True):
    nc = bass.Bass("TRN2", debug=False)
    xT = nc.dram_tensor("xT", [C, B * T], F32R, kind="ExternalInput")
    wq = nc.dram_tensor("wq", [C, CH], F32R, kind="ExternalInput")
    wk = nc.dram_tensor("wk", [C, CH], F32R, kind="ExternalInput")
    wv = nc.dram_tensor("wv", [C, CH], F32R, kind="ExternalInput")
    wo = nc.dram_tensor("wo", [CH, C], F32R, kind="ExternalInput")
    masks = nc.dram_tensor("masks", [4, 128, 512], F32, kind="ExternalInput")
    y = nc.dram_tensor("y", [B * T, C], F32, kind="ExternalOutput")

    KT = C // 128  # 16 contraction tiles
    NT = T // 512  # 4 query tiles of 512 per batch

    with tile.TileContext(nc) as tc, ExitStack() as ctx:
        consts = ctx.enter_context(tc.tile_pool(name="consts", bufs=1))
        qkvp = ctx.enter_context(tc.tile_pool(name="qkv", bufs=1))
        xinp = ctx.enter_context(tc.tile_pool(name="xin", bufs=3))
        wp = ctx.enter_context(tc.tile_pool(name="wtile", bufs=4))
        attnp = ctx.enter_context(tc.tile_pool(name="attn", bufs=2))
        ystp = ctx.enter_context(tc.tile_pool(name="yst", bufs=2))
        smallp = ctx.enter_context(tc.tile_pool(name="small", bufs=2))

        wq_sb = consts.tile([128, KT, CH], F32R, name="wq_sb", tag="wq")
        wk_sb = consts.tile([128, KT, CH], F32R, name="wk_sb", tag="wk")
        wv_sb = consts.tile([128, KT, CH], F32R, name="wv_sb", tag="wv")
        nc.sync.dma_start(wq_sb[:], wq.ap().rearrange("(k p) o -> p k o", p=128))
        nc.sync.dma_start(wk_sb[:], wk.ap().rearrange("(k p) o -> p k o", p=128))
        nc.sync.dma_start(wv_sb[:], wv.ap().rearrange("(k p) o -> p k o", p=128))
        wo_sb = consts.tile([128, H_PER_CORE, C], F32R, name="wo_sb", tag="wo")
        nc.sync.dma_start(wo_sb[:], wo.ap().rearrange("(h p) o -> p h o", p=128))
        mask_sb = consts.tile([128, 4, 512], F32, name="mask_sb", tag="masks")
        for r in range(4):
            nc.sync.dma_start(mask_sb[:, r, :], masks.ap()[r])
        ones_col_f = consts.tile([128, 1], F32, name="ones_col_f", tag="ones_col_f")
        nc.vector.memset(ones_col_f[:], 1.0)
        ones_col = consts.tile([128, 1], F32R, name="ones_col", tag="ones_col")
        nc.scalar.copy(ones_col[:], ones_col_f[:])
        ones_row_f = consts.tile([1, 128], F32, name="ones_row_f", tag="ones_row_f")
        nc.vector.memset(ones_row_f[:], 1.0)
        ones_row = consts.tile([1, 128], F32R, name="ones_row", tag="ones_row")
        nc.scalar.copy(ones_row[:], ones_row_f[:])
        bias_sb = consts.tile([128, 1], F32, name="bias_sb", tag="bias")
        nc.vector.memset(bias_sb[:], ATTN_BIAS)

        xT_re = xT.ap().rearrange("(k p) t -> p k t", p=128)  # (128, KT, B*T)

        for b in range(B):
            q_sb = qkvp.tile([128, H_PER_CORE, T], F32R, name="q_sb", tag="q")
            k_sb = qkvp.tile([128, H_PER_CORE, T], F32R, name="k_sb", tag="k")
            v_sb = qkvp.tile([128, T // 128, CH], F32R, name="v_sb", tag="v")

            # ---- phase 1: projections for this batch's 2048 tokens ----
            with tc.tile_pool(name="ps1", bufs=1, space="PSUM") as pp1:
                for n in range(NT):
                    ps_q = [
                        pp1.tile([128, 512], F32, name=f"ps_q{h}", tag=f"psq{h}")
                        for h in (0, 1)
                    ]
                    ps_k = [
                        pp1.tile([128, 512], F32, name=f"ps_k{h}", tag=f"psk{h}")
                        for h in (0, 1)
                    ]
                    ps_v = [
                        pp1.tile([128, 256], F32, name=f"ps_v{s}", tag=f"psv{s}")
                        for s in range(4)
                    ]
                    for kk in range(KT):
                        xin = xinp.tile([128, 512], F32R, name="xin", tag="xin")
                        nc.sync.dma_start(
                            xin[:], xT_re[:, kk, ds(T * b + 512 * n, 512)]
                        )
                        st, sp = kk == 0, kk == KT - 1
                        for h in (0, 1):
                            nc.tensor.matmul(
                                ps_q[h][:],
                                wq_sb[:, kk, ts(h, 128)],
                                xin[:],
                                start=st,
                                stop=sp,
                            )
                            nc.tensor.matmul(
                                ps_k[h][:],
                                wk_sb[:, kk, ts(h, 128)],
                                xin[:],
                                start=st,
                                stop=sp,
                            )
                        for s in range(4):
                            nc.tensor.matmul(
                                ps_v[s][:],
                                xin[:, ts(s, 128)],
                                wv_sb[:, kk, :],
                                start=st,
                                stop=sp,
                            )
                    for h in (0, 1):
                        # fold the attention scale into q at PSUM drain
                        nc.scalar.mul(q_sb[:, h, ts(n, 512)], ps_q[h][:], SCALE)
                        nc.scalar.copy(k_sb[:, h, ts(n, 512)], ps_k[h][:])
                    for s in range(4):
                        nc.scalar.copy(v_sb[:, 4 * n + s, :], ps_v[s][:])

            # ---- phases 2+3: attention + output projection ----
            with (
                tc.tile_pool(name="ps_s", bufs=2, space="PSUM") as pps,
                tc.tile_pool(name="ps_o", bufs=2, space="PSUM") as ppo,
                tc.tile_pool(name="ps_db", bufs=2, space="PSUM") as ppdb,
                tc.tile_pool(name="ps_y", bufs=2, space="PSUM") as ppy,
            ):
                attn_tiles = {}

                def emit_attention(j, b=b, q_sb=q_sb, k_sb=k_sb, v_sb=v_sb):
                    for hh in (0, 1):
                        nblk = 4 * j + 4
                        po = ppo.tile([128, 512], F32, name="po", tag="po")
                        pd = ppdb.tile([1, 512], F32, name="pd", tag="pdb")
                        for i in range(nblk):
                            psb = pps.tile([128, 512], F32, name="psb", tag="ps")
                            nc.tensor.matmul(
                                psb[:],
                                k_sb[:, hh, ds(128 * i, 128)],
                                q_sb[:, hh, ts(j, 512)],
                                start=True,
                                stop=True,
                            )
                            w_t = wp.tile([128, 512], F32R, name="w_t", tag="w")
                            r = i - 4 * j
                            if r >= 0:  # diagonal block: causal mask
                                # mask*relu(S+b) == relu(mask*(S+b)) for 0/1 mask
                                tmp = wp.tile([128, 512], F32, name="wtmp", tag="wtmp")
                                nc.vector.scalar_tensor_tensor(
                                    tmp[:],
                                    psb[:],
                                    ATTN_BIAS,
                                    mask_sb[:, r, :],
                                    op0=mybir.AluOpType.add,
                                    op1=mybir.AluOpType.mult,
                                )
                                nc.scalar.activation(
                                    w_t[:], tmp[:], AF.Relu, bias=0.0, scale=1.0
                                )
                            else:
                                nc.scalar.activation(
                                    w_t[:], psb[:], AF.Relu, bias=bias_sb[:], scale=1.0
                                )
                            nc.tensor.matmul(
                                po[:],
                                v_sb[:, i, ts(hh, 128)],
                                w_t[:],
                                start=i == 0,
                                stop=i == nblk - 1,
                            )
                            nc.tensor.matmul(
                                pd[:],
                                ones_col[:],
                                w_t[:],
                                start=i == 0,
                                stop=i == nblk - 1,
                            )
                        den = smallp.tile([1, 512], F32, name="den", tag="den")
                        nc.vector.tensor_scalar_add(den[:], pd[:], EPS)
                        rec = smallp.tile([1, 512], F32R, name="rec", tag="rec")
                        with nc.allow_low_precision(
                            reason="f32r reciprocal feeds f32r matmul broadcast"
                        ):
                            nc.vector.reciprocal(rec[:], den[:])
                        pbc = ppdb.tile([128, 512], F32, name="pbc", tag="pdb")
                        nc.tensor.matmul(
                            pbc[:], ones_row[:], rec[:], start=True, stop=True
                        )
                        bc_sb = wp.tile([128, 512], F32, name="bc_sb", tag="bc")
                        nc.scalar.copy(bc_sb[:], pbc[:])
                        nm = wp.tile([128, 512], F32, name="nm", tag="nm")
                        nc.vector.tensor_mul(nm[:], po[:], bc_sb[:])
                        at = attnp.tile(
                            [128, 512], F32R, name=f"at{hh}", tag=f"attn{hh}"
                        )
                        nc.scalar.copy(at[:], nm[:])
                        attn_tiles[(j, hh)] = at

                def emit_outproj(j, b=b):
                    a0 = attn_tiles.pop((j, 0))
                    a1 = attn_tiles.pop((j, 1))
                    for s in range(4):
                        yst = ystp.tile([128, C], F32, name="yst", tag="yst")
                        for ot in range(4):
                            py = ppy.tile([128, 512], F32, name="py", tag="py")
                            nc.tensor.matmul(
                                py[:],
                                a0[:, ts(s, 128)],
                                wo_sb[:, 0, ts(ot, 512)],
                                start=True,
                                stop=False,
                            )
                            nc.tensor.matmul(
                                py[:],
                                a1[:, ts(s, 128)],
                                wo_sb[:, 1, ts(ot, 512)],
                                start=False,
                                stop=True,
                            )
                            nc.scalar.copy(yst[:, ts(ot, 512)], py[:])
                        nc.sync.dma_start(
                            y.ap()[ds(T * b + 512 * j + 128 * s, 128), :], yst[:]
                        )

                emit_attention(0)
                for j in range(1, NT):
                    emit_attention(j)
                    emit_outproj(j - 1)
                emit_outproj(NT - 1)
    if split_waits:
        split_excess_waits(nc)
    return nc


def _host_masks():
    p = np.arange(128, dtype=np.int32)[:, None]
    f = np.arange(512, dtype=np.int32)[None, :]
    return np.stack(
        [(f >= 128 * r + p).astype(np.float32) for r in range(4)], axis=0
    )


def kernel(x, Wq, Wk, Wv, Wo, _trace=False, _trace_kwargs=None):
    global _NC_CACHE
    x = np.ascontiguousarray(np.asarray(x, dtype=np.float32))
    Wq = np.asarray(Wq, dtype=np.float32)
    Wk = np.asarray(Wk, dtype=np.float32)
    Wv = np.asarray(Wv, dtype=np.float32)
    Wo = np.asarray(Wo, dtype=np.float32)

    if _NC_CACHE is None:
        _NC_CACHE = _build()
    nc = _NC_CACHE

    xT = np.ascontiguousarray(x.reshape(B * T, C).T)
    masks = _host_masks()
    in_maps = []
    for c in range(N_CORES):
        sl = slice(CH * c, CH * (c + 1))
        in_maps.append(
            {
                "xT": xT,
                "wq": np.ascontiguousarray(Wq[sl, :].T),
                "wk": np.ascontiguousarray(Wk[sl, :].T),
                "wv": np.ascontiguousarray(Wv[sl, :].T),
                "wo": np.ascontiguousarray(Wo[:, sl].T),
                "masks": masks,
            }
        )

    res = run_bass_kernel_spmd(
        nc,
        in_maps,
        core_ids=list(range(N_CORES)),
        trace=_trace,
        **(_trace_kwargs or {}),
    )
    acc = np.zeros((B * T, C), dtype=np.float64)
    for c in range(N_CORES):
        acc += res.results[c]["y"].astype(np.float64)
    out = acc.astype(np.float32).reshape(B, T, C)
    if _trace:
        return out, res
    return out



# revision 10
# speedup vs baseline: 1.2495x; 1.2495x over previous
"""GhostAttention (B=2, T=2048, C=2048, H=16) on 8 Trainium2 NeuronCores.

Sharding: tensor-parallel over heads (Megatron-style). Core c owns heads
{2c, 2c+1}: it gets the 256 matching rows of Wq/Wk/Wv (column-parallel) and
the 256 matching columns of Wo (row-parallel), computes QKV projections,
masked-relu attention and its partial output projection for both batches,
and writes a full-shape partial y. The host sums the 8 partials.

v2 (bf16 + PE-continuity schedule):
  All matmul operands are bf16 (same PE rate as fp32r, half the SBUF/DMA
  traffic; enables fast DVE ops on 16-bit tiles). The attention scale is
  folded into Wq on the host.
  phase 1: the batch's full x^T lives in SBUF (64KB/partition in bf16), so
           each projection quantity (q/k per head, v per 128-token block)
           accumulates as its own full-bank PSUM group through a 2-bank
           ring -- PSUM allows only one accumulation group per 2KB bank.
           No drain bubbles; drains alternate ACT/DVE; weights arrive in
           4 k-groups so the first matmul starts ~2us in.
  phase 2: S^T blocks (tk=128, tq=512) with the S matmul emitted one block
           ahead of the relu+AV pair; relu (bias folded) alternates between
           ACT and DVE so drain throughput ~2x the PE block rate; diagonal
           blocks get a 0/1 mask multiply on DVE (bf16, 4x mode). AV and a
           ones-column normalizer matmul accumulate per block; the
           reciprocal is broadcast with a rank-1 matmul and applied on DVE
           directly PSUM*PSUM -> bf16 attn tile.
  phase 3: out-projection interleaved between the two head-groups of the
           next j-tile to keep the PE queue deep; PSUM drains alternate
           ACT/DVE; y staged in f32 and DMA'd per 128-token row block.
"""

import math
import sys

if "/opt/trn_rl_repo" not in sys.path:
    sys.path.insert(0, "/opt/trn_rl_repo")

import numpy as np
from contextlib import ExitStack

import concourse.bass as bass
import concourse.mybir as mybir
import concourse.tile as tile
from concourse.bass import ts, ds
from concourse.bass_utils import run_bass_kernel_spmd
from concourse.vector_clock import ScopedClock, VectorClock


def _split_drain_and_barrier(self, tick_clock, wait_clock):
    # This image's walrus caps sem waits per instruction; split the Tile-tail
    # drain waits across single-wait SP nops instead.
    gc = tick_clock.global_clock
    n = len(gc)
    for proc in range(n):
        t = gc[proc]
        if t <= 0:
            continue
        vc = VectorClock([0] * n)
        vc.require_at_least(proc, t)
        nop_inst = self.nc.sync.nop()
        wait_clock.add_sem_waits(nop_inst.ins, ScopedClock({None: vc}))
    self.nc.sync.drain()
    self.nc.all_engine_barrier()
    assert self.sems is not None
    popped = self.nc._tile_sem_poison_stack.pop()
    assert popped is self._sem_poison
    self.nc.clear_and_free_semaphores(list(self.sems.allocated().values()))
    self.nc.all_engine_barrier()


tile.TileContext._drain_and_barrier = _split_drain_and_barrier

_ws_counter = [0]


def split_excess_waits(nc, max_waits=1):
    """Hoist extra per-instruction sem waits onto preceding same-engine NoOps
    (same queue => they execute, and therefore wait, before the instruction)."""
    for fn in nc.m.functions:
        for blk in fn.blocks:
            insts = list(blk.instructions)
            out = []
            changed = False
            for inst in insts:
                si = inst.sync_info
                if si is not None and si.on_wait and len(si.on_wait) > max_waits:
                    waits = list(si.on_wait)
                    extra, keep = waits[:-max_waits], waits[-max_waits:]
                    for s in range(0, len(extra), max_waits):
                        chunk = extra[s : s + max_waits]
                        _ws_counter[0] += 1
                        nop = mybir.InstNoOp(
                            name=f"I-ws-{_ws_counter[0]}",
                            engine=inst.engine,
                            ins=[],
                            outs=[],
                            sync_info=mybir.SyncInfo(on_wait=chunk, on_update=[]),
                        )
                        out.append(nop)
                    inst.sync_info = mybir.SyncInfo(
                        on_wait=keep, on_update=list(si.on_update)
                    )
                    changed = True
                out.append(inst)
            if changed:
                try:
                    blk.instructions[:] = out
                except Exception:
                    blk.set_instructions(out)
    return nc


B, T, C = 2, 2048, 2048
H = 16
HD = C // H  # 128
N_CORES = 8
H_PER_CORE = H // N_CORES  # 2
CH = HD * H_PER_CORE  # 256 channels per core
SCALE = 1.0 / math.sqrt(HD)
ATTN_BIAS = 0.1  # relu(scores - (-0.1)) = relu(scores + 0.1)
EPS = 1e-6

F32 = mybir.dt.float32
F32R = mybir.dt.float32r
BF = mybir.dt.bfloat16
AF = mybir.ActivationFunctionType
ALU = mybir.AluOpType

_NC_CACHE = None

KT = C // 128  # 16 contraction slices
NCH = T // 256  # 8 phase-1 chunks per batch
NT = T // 512  # 4 query tiles of 512 per batch


def _build(split_waits=True):
    nc = bass.Bass("TRN2", debug=False)
    xT = nc.dram_tensor("xT", [C, B * T], BF, kind="ExternalInput")
    wq = nc.dram_tensor("wq", [C, CH], BF, kind="ExternalInput")  # pre-scaled
    wk = nc.dram_tensor("wk", [C, CH], BF, kind="ExternalInput")
    wv = nc.dram_tensor("wv", [C, CH], BF, kind="ExternalInput")
    wo = nc.dram_tensor("wo", [CH, C], BF, kind="ExternalInput")
    masks = nc.dram_tensor("masks", [4, 128, 512], BF, kind="ExternalInput")
    y = nc.dram_tensor("y", [B * T, C], F32, kind="ExternalOutput")

    with tile.TileContext(nc) as tc, ExitStack() as ctx:
        consts = ctx.enter_context(tc.tile_pool(name="consts", bufs=1))
        qkvp = ctx.enter_context(tc.tile_pool(name="qkv", bufs=2))
        xinp = ctx.enter_context(tc.tile_pool(name="xin", bufs=1))
        wp = ctx.enter_context(tc.tile_pool(name="wtile", bufs=4))
        attnp = ctx.enter_context(tc.tile_pool(name="attn", bufs=2))
        ystp = ctx.enter_context(tc.tile_pool(name="yst", bufs=2))
        smallp = ctx.enter_context(tc.tile_pool(name="small", bufs=4))

        wq_sb = consts.tile([128, KT, CH], BF, name="wq_sb", tag="wq")
        wk_sb = consts.tile([128, KT, CH], BF, name="wk_sb", tag="wk")
        wv_sb = consts.tile([128, KT, CH], BF, name="wv_sb", tag="wv")
        # 4 k-slice groups per weight so the first matmuls start early.
        for g in range(4):
            rs = ds(512 * g, 512)
            gs = ds(4 * g, 4)
            for w_d, w_s in ((wq_sb, wq), (wk_sb, wk), (wv_sb, wv)):
                nc.sync.dma_start(
                    w_d[:, gs, :],
                    w_s.ap()[rs, :].rearrange("(k p) o -> p k o", p=128),
                )
        wo_sb = consts.tile([128, H_PER_CORE, C], BF, name="wo_sb", tag="wo")
        nc.sync.dma_start(wo_sb[:], wo.ap().rearrange("(h p) o -> p h o", p=128))
        mask_sb = consts.tile([128, 4, 512], BF, name="mask_sb", tag="masks")
        for r in range(4):
            nc.sync.dma_start(mask_sb[:, r, :], masks.ap()[r])
        ones_col = consts.tile([128, 1], BF, name="ones_col", tag="ones_col")
        nc.vector.memset(ones_col[:], 1.0)
        ones_row_f = consts.tile([1, 128], F32, name="ones_row_f", tag="ones_row_f")
        nc.vector.memset(ones_row_f[:], 1.0)
        ones_row = consts.tile([1, 128], F32R, name="ones_row", tag="ones_row")
        nc.scalar.copy(ones_row[:], ones_row_f[:])
        bias_sb = consts.tile([128, 1], F32, name="bias_sb", tag="bias")
        nc.vector.memset(bias_sb[:], ATTN_BIAS)

        xT_re = xT.ap().rearrange("(k p) t -> p k t", p=128)  # (128, KT, B*T)

        # global ACT/DVE alternation for PSUM drains
        par = [0]

        def drain(dst, src):
            if par[0] % 2 == 0:
                nc.scalar.copy(dst, src)
            else:
                nc.vector.tensor_scalar_add(dst, src, 0.0)
            par[0] += 1

        for b in range(B):
            q_sb = qkvp.tile([128, H_PER_CORE, T], BF, name="q_sb", tag="q")
            k_sb = qkvp.tile([128, H_PER_CORE, T], BF, name="k_sb", tag="k")
            v_sb = qkvp.tile([128, T // 128, CH], BF, name="v_sb", tag="v")

            # ---- phase 1: x resident in SBUF; one PSUM group per bank ----
            xb = xinp.tile([128, KT, T], BF, name="xb", tag="xb")
            for n in range(NT):
                for kk in range(KT):
                    nc.sync.dma_start(
                        xb[:, kk, ts(n, 512)],
                        xT_re[:, kk, ds(T * b + 512 * n, 512)],
                    )
            with tc.tile_pool(name="ps1", bufs=2, space="PSUM") as pp1:
                for n in range(NT):
                    for w_sb, dst in (
                        (wk_sb, k_sb),
                        (wq_sb, q_sb),
                    ):
                        for h in (0, 1):
                            pqk = pp1.tile([128, 512], F32, name="pqk", tag="pqk")
                            for kk in range(KT):
                                nc.tensor.matmul(
                                    pqk[:],
                                    w_sb[:, kk, ts(h, 128)],
                                    xb[:, kk, ds(512 * n, 512)],
                                    start=kk == 0,
                                    stop=kk == KT - 1,
                                )
                            drain(dst[:, h, ts(n, 512)], pqk[:])
                    for tb in range(4):
                        pv = pp1.tile([128, 256], F32, name="pv", tag="pv")
                        for kk in range(KT):
                            nc.tensor.matmul(
                                pv[:],
                                xb[:, kk, ds(512 * n + 128 * tb, 128)],
                                wv_sb[:, kk, :],
                                start=kk == 0,
                                stop=kk == KT - 1,
                            )
                        drain(v_sb[:, 4 * n + tb, :], pv[:])

            # ---- phases 2+3: attention + output projection ----
            with (
                tc.tile_pool(name="ps_s", bufs=2, space="PSUM") as pps,
                tc.tile_pool(name="ps_o", bufs=2, space="PSUM") as ppo,
                tc.tile_pool(name="ps_db", bufs=2, space="PSUM") as ppdb,
                tc.tile_pool(name="ps_y", bufs=2, space="PSUM") as ppy,
            ):
                at_tiles = {}

                def emit_heads(j, hh, q_sb=q_sb, k_sb=k_sb, v_sb=v_sb):
                    nblk = 4 * j + 4
                    po = ppo.tile([128, 512], F32, name="po", tag="po")
                    pd = ppdb.tile([1, 512], F32, name="pd", tag="pdb")
                    psbs = [None] * nblk

                    def s_mm(i):
                        psb = pps.tile([128, 512], F32, name="psb", tag="ps")
                        nc.tensor.matmul(
                            psb[:],
                            k_sb[:, hh, ds(128 * i, 128)],
                            q_sb[:, hh, ts(j, 512)],
                            start=True,
                            stop=True,
                        )
                        psbs[i] = psb

                    s_mm(0)
                    for i in range(nblk):
                        if i + 1 < nblk:
                            s_mm(i + 1)
                        w_t = wp.tile([128, 512], BF, name="w_t", tag="w")
                        psb = psbs[i]
                        if par[0] % 2 == 0:
                            nc.scalar.activation(
                                w_t[:], psb[:], AF.Relu, bias=bias_sb[:], scale=1.0
                            )
                        else:
                            nc.vector.tensor_scalar(
                                w_t[:], psb[:], ATTN_BIAS, 0.0, ALU.add, ALU.max
                            )
                        par[0] += 1
                        r = i - 4 * j
                        if r >= 0:  # diagonal block: causal 0/1 mask
                            nc.vector.tensor_mul(w_t[:], w_t[:], mask_sb[:, r, :])
                        nc.tensor.matmul(
                            po[:],
                            v_sb[:, i, ts(hh, 128)],
                            w_t[:],
                            start=i == 0,
                            stop=i == nblk - 1,
                        )
                        nc.tensor.matmul(
                            pd[:],
                            ones_col[:],
                            w_t[:],
                            start=i == 0,
                            stop=i == nblk - 1,
                        )
                    den = smallp.tile([1, 512], F32, name="den", tag="den")
                    nc.vector.tensor_scalar_add(den[:], pd[:], EPS)
                    rec = smallp.tile([1, 512], F32R, name="rec", tag="rec")
                    with nc.allow_low_precision(
                        reason="f32r reciprocal feeds f32r matmul broadcast"
                    ):
                        nc.vector.reciprocal(rec[:], den[:])
                    pbc = ppdb.tile([128, 512], F32, name="pbc", tag="pdb")
                    nc.tensor.matmul(pbc[:], ones_row[:], rec[:], start=True, stop=True)
                    bc_sb = wp.tile([128, 512], F32, name="bc_sb", tag="bc")
                    nc.scalar.copy(bc_sb[:], pbc[:])
                    at = attnp.tile([128, 512], BF, name=f"at{hh}", tag=f"attn{hh}")
                    nc.vector.tensor_mul(at[:], po[:], bc_sb[:])
                    at_tiles[(j, hh)] = at

                def emit_outproj(j, b=b):
                    a0 = at_tiles.pop((j, 0))
                    a1 = at_tiles.pop((j, 1))
                    for s in range(4):
                        yst = ystp.tile([128, C], F32, name="yst", tag="yst")
                        for ot in range(4):
                            py = ppy.tile([128, 512], F32, name="py", tag="py")
                            nc.tensor.matmul(
                                py[:],
                                a0[:, ts(s, 128)],
                                wo_sb[:, 0, ts(ot, 512)],
                                start=True,
                                stop=False,
                            )
                            nc.tensor.matmul(
                                py[:],
                                a1[:, ts(s, 128)],
                                wo_sb[:, 1, ts(ot, 512)],
                                start=False,
                                stop=True,
                            )
                            drain(yst[:, ts(ot, 512)], py[:])
                        nc.sync.dma_start(
                            y.ap()[ds(T * b + 512 * j + 128 * s, 128), :], yst[:]
                        )

                for j in range(NT):
                    emit_heads(j, 0)
                    if j > 0:
                        emit_outproj(j - 1)
                    emit_heads(j, 1)
                emit_outproj(NT - 1)
    if split_waits:
        split_excess_waits(nc)
    return nc


def _host_masks():
    p = np.arange(128, dtype=np.int32)[:, None]
    f = np.arange(512, dtype=np.int32)[None, :]
    return np.stack(
        [(f >= 128 * r + p).astype(np.float32) for r in range(4)], axis=0
    )


def kernel(x, Wq, Wk, Wv, Wo, _trace=False, _trace_kwargs=None):
    global _NC_CACHE
    import ml_dtypes

    bf16 = ml_dtypes.bfloat16
    x = np.asarray(x, dtype=np.float32)
    Wq = np.asarray(Wq, dtype=np.float32) * SCALE  # fold attention scale
    Wk = np.asarray(Wk, dtype=np.float32)
    Wv = np.asarray(Wv, dtype=np.float32)
    Wo = np.asarray(Wo, dtype=np.float32)

    if _NC_CACHE is None:
        _NC_CACHE = _build()
    nc = _NC_CACHE

    xT = np.ascontiguousarray(x.reshape(B * T, C).T).astype(bf16)
    masks = _host_masks().astype(bf16)
    in_maps = []
    for c in range(N_CORES):
        sl = slice(CH * c, CH * (c + 1))
        in_maps.append(
            {
                "xT": xT,
                "wq": np.ascontiguousarray(Wq[sl, :].T).astype(bf16),
                "wk": np.ascontiguousarray(Wk[sl, :].T).astype(bf16),
                "wv": np.ascontiguousarray(Wv[sl, :].T).astype(bf16),
                "wo": np.ascontiguousarray(Wo[:, sl].T).astype(bf16),
                "masks": masks,
            }
        )

    res = run_bass_kernel_spmd(
        nc,
        in_maps,
        core_ids=list(range(N_CORES)),
        trace=_trace,
        **(_trace_kwargs or {}),
    )
    acc = np.zeros((B * T, C), dtype=np.float64)
    for c in range(N_CORES):
        acc += res.results[c]["y"].astype(np.float64)
    out = acc.astype(np.float32).reshape(B, T, C)
    if _trace:
        return out, res
    return out


# revision 18
# speedup vs baseline: 1.3027x; 1.0426x over previous
"""GhostAttention (B=2, T=2048, C=2048, H=16) on 8 Trainium2 NeuronCores.

Sharding: tensor-parallel over heads (Megatron-style). Core c owns heads
{2c, 2c+1}: it gets the 256 matching rows of Wq/Wk/Wv (column-parallel) and
the 256 matching columns of Wo (row-parallel), computes QKV projections,
masked-relu attention and its partial output projection for both batches,
and writes a full-shape partial y. The host sums the 8 partials.

v2 (bf16 + PE-continuity schedule):
  All matmul operands are bf16 (same PE rate as fp32r, half the SBUF/DMA
  traffic; enables fast DVE ops on 16-bit tiles). The attention scale is
  folded into Wq on the host.
  phase 1: the batch's full x^T lives in SBUF (64KB/partition in bf16), so
           each projection quantity (q/k per head, v per 128-token block)
           accumulates as its own full-bank PSUM group through a 2-bank
           ring -- PSUM allows only one accumulation group per 2KB bank.
           No drain bubbles; drains alternate ACT/DVE; weights arrive in
           4 k-groups so the first matmul starts ~2us in.
  phase 2: S^T blocks (tk=128, tq=512) with the S matmul emitted one block
           ahead of the relu+AV pair; relu (bias folded) alternates between
           ACT and DVE so drain throughput ~2x the PE block rate; diagonal
           blocks get a 0/1 mask multiply on DVE (bf16, 4x mode). AV and a
           ones-column normalizer matmul accumulate per block; the
           reciprocal is broadcast with a rank-1 matmul and applied on DVE
           directly PSUM*PSUM -> bf16 attn tile.
  phase 3: out-projection interleaved between the two head-groups of the
           next j-tile to keep the PE queue deep; PSUM drains alternate
           ACT/DVE; y staged in f32 and DMA'd per 128-token row block.
"""

import math
import sys

if "/opt/trn_rl_repo" not in sys.path:
    sys.path.insert(0, "/opt/trn_rl_repo")

import numpy as np
from contextlib import ExitStack

import concourse.bass as bass
import concourse.mybir as mybir
import concourse.tile as tile
from concourse.bass import ts, ds
from concourse.bass_utils import run_bass_kernel_spmd
from concourse.vector_clock import ScopedClock, VectorClock


def _split_drain_and_barrier(self, tick_clock, wait_clock):
    # This image's walrus caps sem waits per instruction; split the Tile-tail
    # drain waits across single-wait SP nops instead.
    gc = tick_clock.global_clock
    n = len(gc)
    for proc in range(n):
        t = gc[proc]
        if t <= 0:
            continue
        vc = VectorClock([0] * n)
        vc.require_at_least(proc, t)
        nop_inst = self.nc.sync.nop()
        wait_clock.add_sem_waits(nop_inst.ins, ScopedClock({None: vc}))
    self.nc.sync.drain()
    self.nc.all_engine_barrier()
    assert self.sems is not None
    popped = self.nc._tile_sem_poison_stack.pop()
    assert popped is self._sem_poison
    self.nc.clear_and_free_semaphores(list(self.sems.allocated().values()))
    self.nc.all_engine_barrier()


tile.TileContext._drain_and_barrier = _split_drain_and_barrier

_ws_counter = [0]


def split_excess_waits(nc, max_waits=1):
    """Hoist extra per-instruction sem waits onto preceding same-engine NoOps
    (same queue => they execute, and therefore wait, before the instruction)."""
    for fn in nc.m.functions:
        for blk in fn.blocks:
            insts = list(blk.instructions)
            out = []
            changed = False
            for inst in insts:
                si = inst.sync_info
                if si is not None and si.on_wait and len(si.on_wait) > max_waits:
                    waits = list(si.on_wait)
                    extra, keep = waits[:-max_waits], waits[-max_waits:]
                    for s in range(0, len(extra), max_waits):
                        chunk = extra[s : s + max_waits]
                        _ws_counter[0] += 1
                        nop = mybir.InstNoOp(
                            name=f"I-ws-{_ws_counter[0]}",
                            engine=inst.engine,
                            ins=[],
                            outs=[],
                            sync_info=mybir.SyncInfo(on_wait=chunk, on_update=[]),
                        )
                        out.append(nop)
                    inst.sync_info = mybir.SyncInfo(
                        on_wait=keep, on_update=list(si.on_update)
                    )
                    changed = True
                out.append(inst)
            if changed:
                try:
                    blk.instructions[:] = out
                except Exception:
                    blk.set_instructions(out)
    return nc


B, T, C = 2, 2048, 2048
H = 16
HD = C // H  # 128
N_CORES = 8
H_PER_CORE = H // N_CORES  # 2
CH = HD * H_PER_CORE  # 256 channels per core
SCALE = 1.0 / math.sqrt(HD)
ATTN_BIAS = 0.1  # relu(scores - (-0.1)) = relu(scores + 0.1)
EPS = 1e-6

F32 = mybir.dt.float32
F32R = mybir.dt.float32r
BF = mybir.dt.bfloat16
AF = mybir.ActivationFunctionType
ALU = mybir.AluOpType

_NC_CACHE = None

KT = C // 128  # 16 contraction slices
NCH = T // 256  # 8 phase-1 chunks per batch
NT = T // 512  # 4 query tiles of 512 per batch
USE_F32R_J0 = False  # f32r j=0 scores: numerically right in sim, wrong on HW


def _build(split_waits=True):
    nc = bass.Bass("TRN2", debug=False)
    xT = nc.dram_tensor("xT", [C, B * T], BF, kind="ExternalInput")
    wq = nc.dram_tensor("wq", [C, CH], BF, kind="ExternalInput")  # pre-scaled
    wk = nc.dram_tensor("wk", [C, CH], BF, kind="ExternalInput")
    wv = nc.dram_tensor("wv", [C, CH], BF, kind="ExternalInput")
    wo = nc.dram_tensor("wo", [CH, C], BF, kind="ExternalInput")
    masks = nc.dram_tensor("masks", [4, 128, 512], BF, kind="ExternalInput")
    y = nc.dram_tensor("y", [B * T, C], F32, kind="ExternalOutput")

    with tile.TileContext(nc) as tc, ExitStack() as ctx:
        consts = ctx.enter_context(tc.tile_pool(name="consts", bufs=1))
        qkvp = ctx.enter_context(tc.tile_pool(name="qkv", bufs=1))
        xinp = ctx.enter_context(tc.tile_pool(name="xin", bufs=1))
        wp = ctx.enter_context(tc.tile_pool(name="wtile", bufs=4))
        attnp = ctx.enter_context(tc.tile_pool(name="attn", bufs=2))
        ystp = ctx.enter_context(tc.tile_pool(name="yst", bufs=2))
        smallp = ctx.enter_context(tc.tile_pool(name="small", bufs=2))
        posp = ctx.enter_context(tc.tile_pool(name="posb", bufs=2))

        wq_sb = consts.tile([128, KT, CH], BF, name="wq_sb", tag="wq")
        wk_sb = consts.tile([128, KT, CH], BF, name="wk_sb", tag="wk")
        wv_sb = consts.tile([128, KT, CH], BF, name="wv_sb", tag="wv")
        # 4 k-slice groups per weight so the first matmuls start early.
        for g in range(4):
            rs = ds(512 * g, 512)
            gs = ds(4 * g, 4)
            for w_d, w_s in ((wq_sb, wq), (wk_sb, wk), (wv_sb, wv)):
                nc.sync.dma_start(
                    w_d[:, gs, :],
                    w_s.ap()[rs, :].rearrange("(k p) o -> p k o", p=128),
                )
        wo_sb = consts.tile([128, H_PER_CORE, C], BF, name="wo_sb", tag="wo")
        nc.sync.dma_start(wo_sb[:], wo.ap().rearrange("(h p) o -> p h o", p=128))
        mask_sb = consts.tile([128, 4, 512], BF, name="mask_sb", tag="masks")
        for r in range(4):
            nc.sync.dma_start(mask_sb[:, r, :], masks.ap()[r])
        ones_col = consts.tile([128, 1], BF, name="ones_col", tag="ones_col")
        nc.vector.memset(ones_col[:], 1.0)
        ones_row_f = consts.tile([1, 128], F32, name="ones_row_f", tag="ones_row_f")
        nc.vector.memset(ones_row_f[:], 1.0)
        ones_row = consts.tile([1, 128], F32R, name="ones_row", tag="ones_row")
        nc.scalar.copy(ones_row[:], ones_row_f[:])
        bias_sb = consts.tile([128, 1], F32, name="bias_sb", tag="bias")
        nc.vector.memset(bias_sb[:], ATTN_BIAS)

        xT_re = xT.ap().rearrange("(k p) t -> p k t", p=128)  # (128, KT, B*T)

        # global ACT/DVE alternation for PSUM drains
        par = [0]

        def drain(dst, src):
            if par[0] % 2 == 0:
                nc.scalar.copy(dst, src)
            else:
                nc.vector.tensor_scalar_add(dst, src, 0.0)
            par[0] += 1

        relu_ctr = [0]

        def relu_drain(w_t, psb):
            # 3:2 ACT:DVE split -- ACT has more slack than DVE
            if relu_ctr[0] % 5 < 3:
                nc.scalar.activation(
                    w_t[:], psb[:], AF.Relu, bias=bias_sb[:], scale=1.0
                )
            else:
                nc.vector.tensor_scalar(
                    w_t[:], psb[:], ATTN_BIAS, 0.0, ALU.add, ALU.max
                )
            relu_ctr[0] += 1

        for b in range(B):
            q_sb = qkvp.tile([128, H_PER_CORE, T], BF, name="q_sb", tag="q")
            k_sb = qkvp.tile([128, H_PER_CORE, T], BF, name="k_sb", tag="k")
            v_sb = qkvp.tile([128, T // 128, CH], BF, name="v_sb", tag="v")
            # f32r q/k for the first 512 tokens: j=0 rows have few allowed
            # keys, so their normalizer is tiny and score error is amplified.
            q32_sb = qkvp.tile([128, H_PER_CORE, 512], F32R, name="q32", tag="q32")
            k32_sb = qkvp.tile([128, H_PER_CORE, 512], F32R, name="k32", tag="k32")

            # ---- phase 1: x resident in SBUF; one PSUM group per bank ----
            xb = xinp.tile([128, KT, T], BF, name="xb", tag="xb")
            for n in range(NT):
                for kk in range(KT):
                    nc.sync.dma_start(
                        xb[:, kk, ts(n, 512)],
                        xT_re[:, kk, ds(T * b + 512 * n, 512)],
                    )
            with tc.tile_pool(name="ps1", bufs=2, space="PSUM") as pp1:
                for n in range(NT):
                    for w_sb, dst, dst32 in (
                        (wk_sb, k_sb, k32_sb),
                        (wq_sb, q_sb, q32_sb),
                    ):
                        for h in (0, 1):
                            pqk = pp1.tile([128, 512], F32, name="pqk", tag="pqk")
                            for kk in range(KT):
                                nc.tensor.matmul(
                                    pqk[:],
                                    w_sb[:, kk, ts(h, 128)],
                                    xb[:, kk, ds(512 * n, 512)],
                                    start=kk == 0,
                                    stop=kk == KT - 1,
                                )
                            drain(dst[:, h, ts(n, 512)], pqk[:])
                            if n == 0:
                                nc.scalar.copy(dst32[:, h, :], pqk[:])
                    for tb in range(4):
                        pv = pp1.tile([128, 256], F32, name="pv", tag="pv")
                        for kk in range(KT):
                            nc.tensor.matmul(
                                pv[:],
                                xb[:, kk, ds(512 * n + 128 * tb, 128)],
                                wv_sb[:, kk, :],
                                start=kk == 0,
                                stop=kk == KT - 1,
                            )
                        drain(v_sb[:, 4 * n + tb, :], pv[:])

            # ---- phases 2+3: attention + output projection ----
            with (
                tc.tile_pool(name="ps_s", bufs=2, space="PSUM") as pps,
                tc.tile_pool(name="ps_o", bufs=2, space="PSUM") as ppo,
                tc.tile_pool(name="ps_db", bufs=2, space="PSUM") as ppdb,
                tc.tile_pool(name="ps_y", bufs=2, space="PSUM") as ppy,
            ):
                at_tiles = {}
                tails = {}

                def emit_heads(
                    j, hh, q_sb=q_sb, k_sb=k_sb, v_sb=v_sb, q32=q32_sb, k32=k32_sb
                ):
                    nblk = 4 * j + 4
                    po = ppo.tile([128, 512], F32, name="po", tag="po")
                    pd = ppdb.tile([1, 512], F32, name="pd", tag="pdb")
                    psbs = [None] * nblk

                    def s_mm(i):
                        psb = pps.tile([128, 512], F32, name="psb", tag="ps")
                        if USE_F32R_J0 and j == 0:
                            # f32r scores for the small-normalizer rows
                            nc.tensor.matmul(
                                psb[:],
                                k32[:, hh, ds(128 * i, 128)],
                                q32[:, hh, :],
                                start=True,
                                stop=True,
                            )
                        else:
                            nc.tensor.matmul(
                                psb[:],
                                k_sb[:, hh, ds(128 * i, 128)],
                                q_sb[:, hh, ts(j, 512)],
                                start=True,
                                stop=True,
                            )
                        psbs[i] = psb

                    s_mm(0)
                    for i in range(nblk):
                        if i + 1 < nblk:
                            s_mm(i + 1)
                        w_t = wp.tile([128, 512], BF, name="w_t", tag="w")
                        relu_drain(w_t, psbs[i])
                        r = i - 4 * j
                        if r >= 0:  # diagonal block: causal 0/1 mask
                            nc.vector.tensor_mul(w_t[:], w_t[:], mask_sb[:, r, :])
                        nc.tensor.matmul(
                            po[:],
                            v_sb[:, i, ts(hh, 128)],
                            w_t[:],
                            start=i == 0,
                            stop=i == nblk - 1,
                        )
                        nc.tensor.matmul(
                            pd[:],
                            ones_col[:],
                            w_t[:],
                            start=i == 0,
                            stop=i == nblk - 1,
                        )
                    # tail part A (no PE): free the po/pd banks right away and
                    # run the reciprocal off the critical path
                    po_sb = posp.tile([128, 512], F32, name="po_sb", tag="po_sb")
                    nc.scalar.copy(po_sb[:], po[:])
                    den = smallp.tile([1, 512], F32, name="den", tag="den")
                    nc.vector.tensor_scalar_add(den[:], pd[:], EPS)
                    rec_r = smallp.tile([1, 512], F32R, name="rec_r", tag="rec_r")
                    with nc.allow_low_precision(
                        reason="f32r reciprocal feeds f32r matmul broadcast"
                    ):
                        nc.vector.reciprocal(rec_r[:], den[:])
                    tails[(j, hh)] = (po_sb, rec_r)

                def finish_tail(j, hh):
                    # part B: the rank-1 broadcast matmul sits deep in the PE
                    # queue by now, so its input chain is long since resolved
                    po_sb, rec_r = tails.pop((j, hh))
                    pbc = ppdb.tile([128, 512], F32, name="pbc", tag="pdb")
                    nc.tensor.matmul(
                        pbc[:], ones_row[:], rec_r[:], start=True, stop=True
                    )
                    bc_sb = wp.tile([128, 512], F32, name="bc_sb", tag="bc")
                    nc.scalar.copy(bc_sb[:], pbc[:])
                    at = attnp.tile([128, 512], BF, name=f"at{hh}", tag=f"attn{hh}")
                    nc.vector.tensor_mul(at[:], po_sb[:], bc_sb[:])
                    at_tiles[(j, hh)] = at

                def emit_outproj(j, b=b):
                    a0 = at_tiles.pop((j, 0))
                    a1 = at_tiles.pop((j, 1))
                    for s in range(4):
                        yst = ystp.tile([128, C], F32, name="yst", tag="yst")
                        for ot in range(4):
                            py = ppy.tile([128, 512], F32, name="py", tag="py")
                            nc.tensor.matmul(
                                py[:],
                                a0[:, ts(s, 128)],
                                wo_sb[:, 0, ts(ot, 512)],
                                start=True,
                                stop=False,
                            )
                            nc.tensor.matmul(
                                py[:],
                                a1[:, ts(s, 128)],
                                wo_sb[:, 1, ts(ot, 512)],
                                start=False,
                                stop=True,
                            )
                            drain(yst[:, ts(ot, 512)], py[:])
                        nc.sync.dma_start(
                            y.ap()[ds(T * b + 512 * j + 128 * s, 128), :], yst[:]
                        )

                for j in range(NT):
                    emit_heads(j, 0)
                    emit_heads(j, 1)
                    finish_tail(j, 0)
                    if j > 0:
                        emit_outproj(j - 1)
                    finish_tail(j, 1)
                emit_outproj(NT - 1)
    if split_waits:
        split_excess_waits(nc)
    return nc


def _host_masks():
    p = np.arange(128, dtype=np.int32)[:, None]
    f = np.arange(512, dtype=np.int32)[None, :]
    return np.stack(
        [(f >= 128 * r + p).astype(np.float32) for r in range(4)], axis=0
    )


def kernel(x, Wq, Wk, Wv, Wo, _trace=False, _trace_kwargs=None):
    global _NC_CACHE
    import ml_dtypes

    bf16 = ml_dtypes.bfloat16
    x = np.asarray(x, dtype=np.float32)
    Wq = np.asarray(Wq, dtype=np.float32) * SCALE  # fold attention scale
    Wk = np.asarray(Wk, dtype=np.float32)
    Wv = np.asarray(Wv, dtype=np.float32)
    Wo = np.asarray(Wo, dtype=np.float32)

    if _NC_CACHE is None:
        _NC_CACHE = _build()
    nc = _NC_CACHE

    xT = np.ascontiguousarray(x.reshape(B * T, C).T).astype(bf16)
    masks = _host_masks().astype(bf16)
    in_maps = []
    for c in range(N_CORES):
        sl = slice(CH * c, CH * (c + 1))
        in_maps.append(
            {
                "xT": xT,
                "wq": np.ascontiguousarray(Wq[sl, :].T).astype(bf16),
                "wk": np.ascontiguousarray(Wk[sl, :].T).astype(bf16),
                "wv": np.ascontiguousarray(Wv[sl, :].T).astype(bf16),
                "wo": np.ascontiguousarray(Wo[:, sl].T).astype(bf16),
                "masks": masks,
            }
        )

    res = run_bass_kernel_spmd(
        nc,
        in_maps,
        core_ids=list(range(N_CORES)),
        trace=_trace,
        **(_trace_kwargs or {}),
    )
    acc = np.zeros((B * T, C), dtype=np.float64)
    for c in range(N_CORES):
        acc += res.results[c]["y"].astype(np.float64)
    out = acc.astype(np.float32).reshape(B, T, C)
    if _trace:
        return out, res
    return out
